# revision 1
# baseline (speedup 1.0000x reference)
"""Trainium2 Bass kernel for nn_CustomCrossAttentionExt.

Strategy: data-parallel over batch b across 8 NeuronCores. Each core
processes one batch element end-to-end. The global masked std of the
attention logits is computed analytically from per-head Gram matrices
(sum(sim) = qsum.ksum, sum(sim^2) = <K2, Q2> via Y = K2 @ qT), reduced
across cores with one tiny AllReduce, so attention runs in a single
fused pass: sim -> exp -> (* exp(wf)) -> PV -> out projection.

All activations flow "transposed" (feature dim on partitions) so every
matmul has its contraction dim on partitions with base-partition 0.
"""

import functools
import os
import sys

import numpy as np

sys.path.insert(0, "/opt/trn_rl_repo")

import ml_dtypes

import concourse.bass as bass
import concourse.tile as tile
from concourse import bacc, mybir
from concourse.bass_utils import run_bass_kernel_spmd
from concourse.masks import make_identity

B, N, J = 8, 4096, 308
QD, CD, H, DH = 640, 768, 8, 80
INNER = H * DH
SCALE = DH ** -0.5

F32 = mybir.dt.float32
F32R = mybir.dt.float32r
BF16 = mybir.dt.bfloat16
AF = mybir.ActivationFunctionType
ALU = mybir.AluOpType

JC = [(0, 128), (128, 128), (256, 52)]          # j chunks of 308
KQ = [(0, 128), (128, 128), (256, 64)]          # k chunks of 320
MQ = [(0, 128), (128, 128), (256, 64)]          # m chunks of 320
NI = 512                                        # i-chunk size
NIT = N // NI                                   # 8 i-chunks
FSPLIT = [(0, 384), (384, 256)]                 # N-splits of 640 (>=256 for f32r)
VSPLIT = [(0, 320), (320, 320)]                 # v N-split aligned to head groups


def _r(ap):
    return ap.bitcast(F32R)


def _emit(tc, nc, io):
    """Emit the whole per-core program under TileContext tc."""
    from contextlib import ExitStack

    ctx = ExitStack()
    consts = ctx.enter_context(tc.tile_pool(name="consts", bufs=1))
    dram = ctx.enter_context(tc.tile_pool(name="dram", bufs=1, space="DRAM"))

    # ---------- persistent constants ----------
    wq = consts.tile([128, 5, QD], F32R, tag="wq", name="wq")
    nc.sync.dma_start(out=wq, in_=io["Wq"].rearrange("(c p) n -> p c n", p=128))
    w2q = consts.tile([128, 3, QD], F32R, tag="w2q", name="w2q")
    for kc, (k0, ksz) in enumerate(KQ):
        nc.sync.dma_start(out=w2q[0:ksz, kc, :], in_=io["W2q"][k0:k0 + ksz, :])
    w1 = consts.tile([128, 5, 320], F32R, tag="w1", name="w1")
    nc.sync.dma_start(out=w1, in_=io["W1"].rearrange("(c p) n -> p c n", p=128))
    woh = consts.tile([80, 8, QD], F32R, tag="woh", name="woh")
    nc.sync.dma_start(out=woh, in_=io["Wo"].rearrange("(h d) n -> d h n", d=DH))
    qsv = consts.tile([128, 5], F32, tag="qsv", name="qsv")
    nc.sync.dma_start(out=qsv, in_=io["qsv"].rearrange("(c p) -> p c", p=128))
    peb1 = consts.tile([128, 3], F32, tag="peb1", name="peb1")
    for kc, (k0, ksz) in enumerate(KQ):
        nc.sync.dma_start(out=peb1[0:ksz, kc:kc + 1], in_=io["peb1"][k0:k0 + ksz].rearrange("(p one) -> p one", one=1))
    cwq = consts.tile([80, 8], F32, tag="cwq", name="cwq")
    nc.sync.dma_start(out=cwq, in_=io["cwq"].rearrange("(h d) -> d h", d=DH))
    sc = consts.tile([1, 8], F32, tag="sc", name="sc")
    nc.sync.dma_start(out=sc, in_=io["sc"].rearrange("(one n) -> one n", one=1))
    bo_bc = consts.tile([128, QD], F32, tag="bo_bc", name="bo_bc")
    bo_ap = io["bo"]
    nc.gpsimd.dma_start(out=bo_bc, in_=bass.AP(tensor=bo_ap.tensor, offset=bo_ap.offset, ap=[[0, 128]] + list(bo_ap.ap)))
    kmask_t = []
    for jci, (j0, jsz) in enumerate(JC):
        t = consts.tile([jsz, 1], F32, tag=f"kmask{jci}", name=f"kmask{jci}")
        nc.sync.dma_start(out=t, in_=io["kmaskv"][j0:j0 + jsz].rearrange("(p one) -> p one", one=1))
        kmask_t.append(t)
    ident = consts.tile([128, 128], BF16, tag="ident", name="ident")
    make_identity(nc, ident)
    ones80 = consts.tile([80, 1], F32, tag="ones80", name="ones80")
    nc.vector.memset(ones80, 1.0)
    onesR = consts.tile([97, 80], F32R, tag="onesR", name="onesR")
    nc.vector.memset(onesR[96:97, :].bitcast(F32), 1.0)

    # persistent per-head tensors
    qts = [consts.tile([80, N], BF16, tag=f"qts{h}", name=f"qts{h}") for h in range(H)]
    kts = [consts.tile([80, J], BF16, tag=f"kts{h}", name=f"kts{h}") for h in range(H)]
    k2 = [consts.tile([80, 80], BF16, tag=f"k2{h}", name=f"k2{h}") for h in range(H)]
    ksum = consts.tile([80, 8], F32, tag="ksum", name="ksum")
    qsum = consts.tile([80, 8], F32, tag="qsum", name="qsum")
    part = consts.tile([80, 2], F32, tag="part", name="part")
    nc.vector.memset(part, 0.0)
    ss64 = consts.tile([80, NIT * H], F32, tag="ss64", name="ss64")
    h1sums = consts.tile([128, 3, NIT], F32, tag="h1sums", name="h1sums")
    va = []
    for jci, (j0, jsz) in enumerate(JC):
        t = consts.tile([jsz, 8, 97], BF16, tag=f"va{jci}", name=f"va{jci}")
        nc.vector.memset(t, 0.0)
        va.append(t)
    wfb_bc = consts.tile([128, 1], F32, tag="wfb_bc", name="wfb_bc")

    # ---------- phase A: k-side ----------
    with tc.tile_pool(name="kside", bufs=1) as kside, \
         tc.tile_pool(name="psA", bufs=2, space="PSUM") as psA, \
         tc.tile_pool(name="psA2", bufs=1, space="PSUM") as psA2:
        t1 = kside.tile([128, 6, 384], F32R, tag="t1", name="t1")
        nc.sync.dma_start(out=t1, in_=io["T1"].rearrange("(c p) n -> p c n", p=128))
        wk = kside.tile([128, 6, INNER], F32R, tag="wk", name="wk")
        nc.sync.dma_start(out=wk, in_=io["Wk"].rearrange("(c p) n -> p c n", p=128))
        w2k = kside.tile([128, 3, INNER], F32R, tag="w2k", name="w2k")
        nc.sync.dma_start(out=w2k, in_=io["W2k"].rearrange("(c p) n -> p c n", p=128))
        wv = kside.tile([128, 6, INNER], F32R, tag="wv", name="wv")
        nc.sync.dma_start(out=wv, in_=io["Wv"].rearrange("(c p) n -> p c n", p=128))
        ekt = kside.tile([128, 6, J], F32R, tag="ekt", name="ekt")
        nc.sync.dma_start(out=ekt, in_=io["ekT"].rearrange("(c p) j -> p c j", p=128))
        ekat = kside.tile([128, 6, J], F32R, tag="ekat", name="ekat")
        nc.sync.dma_start(out=ekat, in_=io["ekAT"].rearrange("(c p) j -> p c j", p=128))
        embst = kside.tile([128, 6, J], F32R, tag="embst", name="embst")
        nc.sync.dma_start(out=embst, in_=io["embsT"].rearrange("(c p) j -> p c j", p=128))
        tb1 = kside.tile([128, 3], F32, tag="tb1", name="tb1")
        nc.sync.dma_start(out=tb1, in_=io["tb1"].rearrange("(c p) -> p c", p=128))

        # hT = gelu(T1.T @ ekT + tb1)   [384, J]
        ht = kside.tile([128, 3, J], F32R, tag="ht", name="ht")
        for mc in range(3):
            ps = psA.tile([128, J], F32, tag="htps", name="htps")
            for kc in range(6):
                nc.tensor.matmul(ps, _r(t1[:, kc, mc * 128:(mc + 1) * 128]), ekt[:, kc, :],
                                 start=(kc == 0), stop=(kc == 5))
            nc.scalar.activation(ht[:, mc, :], ps, AF.Gelu, bias=tb1[:, mc:mc + 1])

        # kT_h = (Wk.T @ ekAT + W2k.T @ hT) per head  [80, J]
        for h in range(H):
            ps = psA.tile([80, J], F32, tag="ktps", name="ktps")
            for kc in range(6):
                nc.tensor.matmul(ps, _r(wk[:, kc, h * DH:(h + 1) * DH]), ekat[:, kc, :],
                                 start=(kc == 0), stop=False)
            for kc, (k0, ksz) in enumerate(KQ[:3]):
                if k0 >= 384:
                    break
            for kc, (k0, ksz) in enumerate([(0, 128), (128, 128), (256, 128)]):
                nc.tensor.matmul(ps, _r(w2k[:, kc, h * DH:(h + 1) * DH]), ht[:, kc, :],
                                 start=False, stop=(kc == 2))
            nc.vector.tensor_copy(kts[h], ps)

        # v = embs @ Wv  -> va (masked, with keymask col at 96)
        for jci, (j0, jsz) in enumerate(JC):
            for (n0, nsz) in VSPLIT:
                ps = psA2.tile([jsz, 320], F32, tag="vps", name="vps")
                for kc in range(6):
                    nc.tensor.matmul(ps, embst[:, kc, j0:j0 + jsz], wv[:, kc, n0:n0 + nsz],
                                     start=(kc == 0), stop=(kc == 5))
                for h in range(n0 // DH, (n0 + nsz) // DH):
                    nc.vector.tensor_scalar(va[jci][:, h, 0:80], ps[:, h * DH - n0:(h + 1) * DH - n0],
                                            kmask_t[jci], None, op0=ALU.mult)
            for h in range(H):
                nc.vector.tensor_copy(va[jci][:, h, 96:97], kmask_t[jci])

        # masked k gram: K2_h and ksum_h
        for h in range(H):
            kms = []
            for jci, (j0, jsz) in enumerate(JC):
                tp = psA2.tile([jsz, 80], BF16, tag="ktr", name="ktr")
                nc.tensor.transpose(tp, kts[h][:, j0:j0 + jsz], ident[0:80, 0:80])
                km = kside.tile([jsz, 81], BF16, tag=f"km{jci}", name=f"km{jci}")
                nc.vector.tensor_scalar(km[:, 0:80], tp, kmask_t[jci], None, op0=ALU.mult)
                nc.vector.tensor_copy(km[:, 80:81], kmask_t[jci])
                kms.append(km)
            gps = psA2.tile([81, 81], F32, tag="gram", name="gram")
            for jci, (j0, jsz) in enumerate(JC):
                nc.tensor.matmul(gps, kms[jci], kms[jci], start=(jci == 0), stop=(jci == 2))
            nc.vector.tensor_copy(k2[h], gps[0:80, 0:80])
            nc.vector.tensor_copy(ksum[:, h:h + 1], gps[0:80, 80:81])

    if os.environ.get("KSTAGE", "full") == "A":
        dbg = consts.tile([1, 1], F32, tag="dbg", name="dbg")
        nc.vector.tensor_copy(dbg, ksum[0:1, 0:1])
        nc.sync.dma_start(out=io["out"][0:1, 0:1], in_=dbg)
        ctx.close()
        return
    # ---------- phase B: q-side (pm MLP folded) + stats ----------
    with tc.tile_pool(name="bwork", bufs=2) as bwork, \
         tc.tile_pool(name="bscr", bufs=2) as bscr, \
         tc.tile_pool(name="psB1", bufs=2, space="PSUM") as psB1, \
         tc.tile_pool(name="psB2", bufs=3, space="PSUM") as psB2, \
         tc.tile_pool(name="psY", bufs=2, space="PSUM") as psY:
        xt_r = io["xT"].rearrange("(c p) i -> p c i", p=128)
        for it in range(NIT):
            i0 = it * NI
            xt = bwork.tile([128, 5, NI], F32R, tag="xt", name="xt")
            nc.sync.dma_start(out=xt, in_=xt_r[:, :, i0:i0 + NI])
            h1 = bwork.tile([128, 3, NI], F32R, tag="h1", name="h1")
            for mc, (m0, msz) in enumerate(MQ):
                ps = psB1.tile([msz, NI], F32, tag="h1ps", name="h1ps")
                for kc in range(5):
                    nc.tensor.matmul(ps, w1[:, kc, m0:m0 + msz], xt[:, kc, :],
                                     start=(kc == 0), stop=(kc == 4))
                if os.environ.get("KSKIP_ACC") == "1":
                    nc.scalar.activation(h1[0:msz, mc, :], ps, AF.Gelu,
                                         bias=peb1[0:msz, mc:mc + 1])
                else:
                    nc.scalar.activation(h1[0:msz, mc, :], ps, AF.Gelu,
                                         bias=peb1[0:msz, mc:mc + 1],
                                         accum_out=h1sums[0:msz, mc, it:it + 1])
            for h in range(H):
                ps = psB2.tile([80, NI], F32, tag="qtps", name="qtps")
                for kc in range(5):
                    nc.tensor.matmul(ps, _r(wq[:, kc, h * DH:(h + 1) * DH]), xt[:, kc, :],
                                     start=(kc == 0), stop=False)
                for kc, (k0, ksz) in enumerate(KQ):
                    nc.tensor.matmul(ps, _r(w2q[0:ksz, kc, h * DH:(h + 1) * DH]), h1[0:ksz, kc, :],
                                     start=False, stop=(kc == 2))
                nc.vector.tensor_scalar(qts[h][:, i0:i0 + NI], ps, cwq[:, h:h + 1], SCALE,
                                        op0=ALU.add, op1=ALU.mult)
                # stats: Y = K2 @ qts, SS += <Y, qts>
                if os.environ.get("KSKIP_TTR") != "1":
                    yps = psY.tile([80, NI], F32, tag="yps", name="yps")
                    nc.tensor.matmul(yps, k2[h], qts[h][:, i0:i0 + NI], start=True, stop=True)
                    scr = bscr.tile([80, NI], F32, tag="ttr", name="ttr")
                    nc.vector.tensor_mul(scr, yps, qts[h][:, i0:i0 + NI])
                    nc.vector.tensor_reduce(out=ss64[:, it * H + h:it * H + h + 1],
                                            in_=scr, axis=mybir.AxisListType.X, op=ALU.add)

    if os.environ.get("KSTAGE", "full") == "B":
        dbg = consts.tile([1, 1], F32, tag="dbg", name="dbg")
        nc.vector.tensor_copy(dbg, part[0:1, 0:1])
        nc.sync.dma_start(out=io["out"][0:1, 0:1], in_=dbg)
        ctx.close()
        return
    # ---------- stats epilogue + collective ----------
    with tc.tile_pool(name="psQ", bufs=2, space="PSUM") as psQ:
        nc.vector.tensor_reduce(out=part[:, 0:1], in_=ss64, axis=mybir.AxisListType.X, op=ALU.add)
        h1sum = consts.tile([128, 3], F32, tag="h1sum", name="h1sum")
        nc.vector.tensor_reduce(out=h1sum, in_=h1sums, axis=mybir.AxisListType.X, op=ALU.add)
        for h in range(H):
            qps = psQ.tile([80, 1], F32, tag="qps", name="qps")
            for kc in range(5):
                nc.tensor.matmul(qps, wq[:, kc, h * DH:(h + 1) * DH].bitcast(F32), qsv[:, kc:kc + 1],
                                 start=(kc == 0), stop=False)
            for kc, (k0, ksz) in enumerate(KQ):
                nc.tensor.matmul(qps, w2q[0:ksz, kc, h * DH:(h + 1) * DH].bitcast(F32),
                                 h1sum[0:ksz, kc:kc + 1],
                                 start=False, stop=(kc == 2))
            nc.vector.tensor_copy(qsum[:, h:h + 1], qps)
        scr2 = consts.tile([80, 8], F32, tag="scr2", name="scr2")
        nc.vector.tensor_mul(scr2, qsum, ksum)
        nc.vector.tensor_reduce(out=part[:, 1:2], in_=scr2, axis=mybir.AxisListType.X, op=ALU.add)
        pp = psQ.tile([2, 1], F32, tag="pp", name="pp")
        nc.tensor.matmul(pp, part, ones80, start=True, stop=True)
        ppsb = consts.tile([2, 1], F32, tag="ppsb", name="ppsb")
        nc.vector.tensor_copy(ppsb, pp)

        if os.environ.get("KSTAGE", "full") == "B2":
            dbg = consts.tile([1, 1], F32, tag="dbg", name="dbg")
            nc.vector.tensor_copy(dbg, ppsb[0:1, 0:1])
            nc.sync.dma_start(out=io["out"][0:1, 0:1], in_=dbg)
            ctx.close()
            return
        cc_in = dram.tile([1, 8], F32, tag="cc_in", name="cc_in")
        cc_out = dram.tile([1, 8], F32, tag="cc_out", name="cc_out")
        z8 = consts.tile([1, 8], F32, tag="z8", name="z8")
        nc.vector.memset(z8, 0.0)
        nc.sync.dma_start(out=cc_in, in_=z8)
        nc.sync.dma_start(out=cc_in[0:1, 0:2], in_=ppsb.rearrange("p one -> one p"))
        nc.gpsimd.collective_compute(
            "AllReduce", ALU.add,
            replica_groups=[list(range(B))],
            ins=[cc_in.opt()], outs=[cc_out.opt()])
        stats = consts.tile([1, 8], F32, tag="stats", name="stats")
        nc.sync.dma_start(out=stats, in_=cc_out)

        # wf1 = strength * sqrt((SS - S*S/cnt) / (cnt-1))  (all SCALE-folded)
        t0 = consts.tile([1, 4], F32, tag="t0", name="t0")
        nc.vector.tensor_tensor(t0[:, 0:1], stats[:, 1:2], stats[:, 1:2], ALU.mult)
        nc.vector.tensor_tensor(t0[:, 1:2], t0[:, 0:1], sc[:, 0:1], ALU.mult)
        nc.vector.tensor_tensor(t0[:, 2:3], stats[:, 0:1], t0[:, 1:2], ALU.subtract)
        nc.vector.tensor_tensor(t0[:, 3:4], t0[:, 2:3], sc[:, 1:2], ALU.mult)
        ln3 = consts.tile([1, 2], F32, tag="ln3", name="ln3")
        nc.scalar.activation(ln3[:, 0:1], t0[:, 3:4], AF.Ln)
        nc.scalar.activation(ln3[:, 1:2], ln3[:, 0:1], AF.Exp, scale=0.5)
        wf1 = consts.tile([1, 1], F32, tag="wf1", name="wf1")
        nc.vector.tensor_tensor(wf1, ln3[:, 1:2], sc[:, 2:3], ALU.mult)
        wf_dram = dram.tile([1, 1], F32, tag="wf_dram", name="wf_dram")
        nc.sync.dma_start(out=wf_dram, in_=wf1)
        nc.gpsimd.dma_start(out=wfb_bc, in_=bass.AP(tensor=wf_dram.tensor, offset=wf_dram.offset,
                                                  ap=[[0, 128], [1, 1]]))

    # ---------- phase D: attention ----------
    if os.environ.get("KSTAGE", "full") == "AB":
        dbg = consts.tile([1, 1], F32, tag="dbg", name="dbg")
        nc.vector.tensor_copy(dbg, wf1)
        nc.sync.dma_start(out=io["out"][0:1, 0:1], in_=dbg)
        ctx.close()
        return
    with tc.tile_pool(name="dwork", bufs=2) as dwork, \
         tc.tile_pool(name="aowork", bufs=10) as aowork, \
         tc.tile_pool(name="psDs", bufs=3, space="PSUM") as psDs, \
         tc.tile_pool(name="psDao", bufs=2, space="PSUM") as psDao, \
         tc.tile_pool(name="psDr", bufs=1, space="PSUM") as psDr, \
         tc.tile_pool(name="psDf", bufs=2, space="PSUM") as psDf:
        for it in range(NIT):
            i0 = it * NI
            mt = dwork.tile([128, 3, NI], BF16, tag="mt", name="mt")
            et = dwork.tile([128, 3, NI], BF16, tag="et", name="et")
            for jci, (j0, jsz) in enumerate(JC):
                nc.sync.dma_start(out=mt[0:jsz, jci, :], in_=io["maskT"][j0:j0 + jsz, i0:i0 + NI])
                nc.scalar.activation(et[0:jsz, jci, :], mt[0:jsz, jci, :], AF.Exp,
                                     scale=wfb_bc[0:jsz, 0:1])
            ao_tiles = []
            for h in range(H):
                ept = dwork.tile([128, 3, NI], BF16, tag="ept", name="ept")
                ee = dwork.tile([128, 3, NI], BF16, tag="ee", name="ee")
                aops = psDao.tile([97, NI], F32, tag="aops", name="aops")
                for jci, (j0, jsz) in enumerate(JC):
                    sps = psDs.tile([128, NI], F32, tag="sps", name="sps")
                    nc.tensor.matmul(sps[0:jsz, :], kts[h][:, j0:j0 + jsz], qts[h][:, i0:i0 + NI],
                                     start=True, stop=True)
                    nc.scalar.activation(ept[0:jsz, jci, :], sps[0:jsz, :], AF.Exp)
                    nc.vector.tensor_mul(ee[0:jsz, jci, :], ept[0:jsz, jci, :], et[0:jsz, jci, :])
                    nc.tensor.matmul(aops, va[jci][:, h, :], ee[0:jsz, jci, :],
                                     start=(jci == 0), stop=(jci == 2))
                rec = dwork.tile([97, NI], F32R, tag="rec", name="rec")
                with nc.allow_low_precision("f32r reciprocal feeding broadcast matmul"):
                    nc.vector.reciprocal(rec[96:97, :], aops[96:97, :])
                rps = psDr.tile([80, NI], F32, tag="rps", name="rps")
                nc.tensor.matmul(rps, onesR[96:97, :], rec[96:97, :],
                                 start=True, stop=True, tile_position=(96, 0))
                rsb = dwork.tile([80, NI], F32, tag="rsb", name="rsb")
                nc.scalar.copy(rsb, rps)
                ao = aowork.tile([80, NI], F32R, tag="ao", name="ao")
                nc.vector.tensor_tensor(ao, aops[0:80, :], rsb, ALU.mult)
                ao_tiles.append(ao)
            for isub in range(NI // 128):
                osb = dwork.tile([128, QD], F32, tag="osb", name="osb")
                for (n0, nsz) in FSPLIT:
                    fps = psDf.tile([128, nsz], F32, tag="fin", name="fin")
                    for h in range(H):
                        nc.tensor.matmul(fps, _r(ao_tiles[h][:, isub * 128:(isub + 1) * 128]),
                                         woh[:, h, n0:n0 + nsz],
                                         start=(h == 0), stop=(h == H - 1))
                    nc.scalar.copy(osb[:, n0:n0 + nsz], fps)
                osb2 = dwork.tile([128, QD], F32, tag="osb2", name="osb2")
                nc.gpsimd.tensor_add(osb2, osb, bo_bc)
                nc.sync.dma_start(out=io["out"][i0 + isub * 128:i0 + (isub + 1) * 128, :], in_=osb2)

    ctx.close()


@functools.lru_cache(maxsize=1)
def _build():
    nc = bacc.Bacc("TRN2", target_bir_lowering=False, debug=False,
                   enable_asserts=False, num_devices=B)
    io = {}

    def inp(name, shape, dtype=F32):
        io[name] = nc.dram_tensor(name, list(shape), dtype, kind="ExternalInput").ap()

    inp("xT", (QD, N), F32R)
    inp("maskT", (J, N), BF16)
    inp("ekT", (CD, J), F32R)
    inp("ekAT", (CD, J), F32R)
    inp("embsT", (CD, J), F32R)
    inp("kmaskv", (J,))
    inp("qsv", (QD,))
    inp("peb1", (320,))
    inp("cwq", (QD,))
    inp("Wq", (QD, QD), F32R)
    inp("W2q", (320, QD), F32R)
    inp("W1", (QD, 320), F32R)
    inp("Wo", (INNER, QD), F32R)
    inp("Wk", (CD, INNER), F32R)
    inp("W2k", (384, INNER), F32R)
    inp("Wv", (CD, INNER), F32R)
    inp("T1", (CD, 384), F32R)
    inp("tb1", (384,))
    inp("bo", (QD,))
    inp("sc", (8,))
    io["out"] = nc.dram_tensor("out", [N, QD], F32, kind="ExternalOutput").ap()

    with tile.TileContext(nc) as tc:
        _emit(tc, nc, io)
    nc.compile()
    return nc


def _host_prep(inputs):
    """Compute per-core input maps from full inputs."""
    f32 = np.float32
    g = {k: np.asarray(v) for k, v in inputs.items()}
    x = g["x"].astype(f32)
    embs = g["embs"].astype(f32)
    progress = g["progress"].astype(f32)
    mask = g["cross_attn_mask"].astype(f32)
    strength = f32(g["strength"])
    ct = g["captiontypes"]

    tte = g["tt_emb"][np.clip(ct, 0, None)]                     # [B,J,CD]
    kmask = (ct >= 0).astype(f32)                               # [B,J]
    cnt = f32(kmask.sum() * (H * N))

    # progress embedding (host, tiny)
    pe_h = np.maximum(progress[:, None] * g["pe_w1"][0][None, :] + g["pe_b1"][None, :], 0.0)
    pe = pe_h @ g["pe_w2"] + g["pe_b2"]                         # [B,QD]
    c = pe * g["pg_gA"][None, :] + (g["pg_gB"] * g["pm_b2"])[None, :]   # [B,QD]

    W2q = (g["pm_w2"] * g["pg_gB"][None, :]).astype(f32) @ g["Wq"]
    W2k = (g["tt_w2"] * g["tt_gB"][None, :]).astype(f32) @ g["Wk"]

    shared = {
        "Wq": np.ascontiguousarray(g["Wq"], f32),
        "W2q": np.ascontiguousarray(W2q, f32),
        "W1": np.ascontiguousarray(g["pm_w1"], f32),
        "Wo": np.ascontiguousarray(g["Wo"], f32),
        "Wk": np.ascontiguousarray(g["Wk"], f32),
        "W2k": np.ascontiguousarray(W2k, f32),
        "Wv": np.ascontiguousarray(g["Wv"], f32),
        "T1": np.ascontiguousarray(g["tt_w1"], f32),
        "tb1": np.ascontiguousarray(g["tt_b1"], f32),
        "bo": np.ascontiguousarray(g["bo"], f32),
        "sc": np.array([(DH ** -1.0) / cnt, 1.0 / (cnt - 1.0), strength, 0, 0, 0, 0, 0], f32),
    }

    ekA = embs + tte * g["tt_gA"][None, None, :] + (g["tt_b2"] * g["tt_gB"])[None, None, :]
    ek = embs + tte

    in_maps = []
    for b in range(B):
        m = dict(shared)
        m["xT"] = np.ascontiguousarray(x[b].T, f32)
        m["maskT"] = np.ascontiguousarray(mask[b].T).astype(ml_dtypes.bfloat16)
        m["ekT"] = np.ascontiguousarray(ek[b].T, f32)
        m["ekAT"] = np.ascontiguousarray(ekA[b].T, f32)
        m["embsT"] = np.ascontiguousarray(embs[b].T, f32)
        m["kmaskv"] = np.ascontiguousarray(kmask[b], f32)
        m["qsv"] = np.ascontiguousarray(x[b].sum(0) + N * c[b], f32)
        m["peb1"] = np.ascontiguousarray(pe[b] @ g["pm_w1"] + g["pm_b1"], f32)
        m["cwq"] = np.ascontiguousarray(c[b] @ g["Wq"], f32)
        in_maps.append(m)
    return in_maps


def kernel(**inputs):
    in_maps = _host_prep(inputs)
    nc = _build()
    res = run_bass_kernel_spmd(nc, in_maps, list(range(B)))
    out = np.stack([res.results[b]["out"] for b in range(B)], axis=0)
    return out.astype(np.float32)

